# revision 3
# baseline (speedup 1.0000x reference)
"""DCT-compressed attention (nn_DCTAttentionIdeal) on 8 Trainium2 NeuronCores.

Math (per head, reference ordering):
    S    = (Q*s) @ (K*mask*s)^T with s = D**-0.25             [N,N]
    atn  = softmax(S, axis=-1)
    Vd   = Qd @ (V*mask)                                  [M,D]
    out  = Qd^T @ ((Qd @ atn @ Qd^T) @ Vd)                [N,D]

Kernel reshaping used here (exact in real arithmetic):
  - softmax max-subtraction is skipped (scores ~ N(0,1) after the 1/8 scale,
    folded into the Exp activation's `scale`).
  - the per-row 1/denom is folded into the DCT columns:
        A1^T[k,m] = sum_q exp(S)[q,k] * (Qd^T[q,m]/denom[q])
    so the [N,N] exp matrix is consumed unnormalized straight out of PSUM.
  - final contraction is reassociated: out = Qd^T @ (G @ Vd), G = A1 @ Qd^T.

Sharding: batch*heads (2*16=32) split 4-per-core across 8 cores; Q_dct
replicated; no cross-core communication.  Host pre-transposes Q and Q_dct
(pure layout), everything else (masking, K transpose, softmax, DCT algebra)
runs on-device.
"""

import numpy as np
import ml_dtypes

import concourse.bass as bass
import concourse.tile as tile
from concourse import bacc, mybir
from concourse import bass_utils
from concourse.masks import make_identity

F32 = mybir.dt.float32
BF16 = mybir.dt.bfloat16
NPBF16 = ml_dtypes.bfloat16
AF = mybir.ActivationFunctionType
ALU = mybir.AluOpType
AX = mybir.AxisListType

B, H, N, D, M = 2, 16, 2048, 64, 256
NCORES = 8
HPC = (B * H) // NCORES  # heads per core = 4
NT = N // 128            # 16 (q and k 128-blocks)
MT = M // 128            # 2
KC = N // 512            # 4 (512-wide score chunks)


def _emit(tc, ctx, io):
    nc = tc.nc
    P = 128

    sh = ctx.enter_context(tc.tile_pool(name="shared", bufs=1))
    exp_pool = ctx.enter_context(tc.tile_pool(name="exp", bufs=18))
    kt_pool = ctx.enter_context(tc.tile_pool(name="ktr", bufs=2))
    qt_pool = ctx.enter_context(tc.tile_pool(name="qtr", bufs=2))
    v_pool = ctx.enter_context(tc.tile_pool(name="vnat", bufs=2))
    cq_pool = ctx.enter_context(tc.tile_pool(name="cq", bufs=2))
    a1_pool = ctx.enter_context(tc.tile_pool(name="a1t", bufs=2))
    gt_pool = ctx.enter_context(tc.tile_pool(name="gt", bufs=2))
    vd_pool = ctx.enter_context(tc.tile_pool(name="vd", bufs=2))
    y_pool = ctx.enter_context(tc.tile_pool(name="y", bufs=2))
    ost_pool = ctx.enter_context(tc.tile_pool(name="ost", bufs=4))
    kld_pool = ctx.enter_context(tc.tile_pool(name="kld", bufs=4))
    msk_pool = ctx.enter_context(tc.tile_pool(name="msk", bufs=2))
    st_pool = ctx.enter_context(tc.tile_pool(name="stats", bufs=6))

    ps_s = ctx.enter_context(tc.tile_pool(name="ps_s", bufs=3, space="PSUM"))
    ps_a1 = ctx.enter_context(tc.tile_pool(name="ps_a1", bufs=2, space="PSUM"))
    ps_gt = ctx.enter_context(tc.tile_pool(name="ps_gt", bufs=1, space="PSUM"))
    ps_m = ctx.enter_context(tc.tile_pool(name="ps_m", bufs=2, space="PSUM"))

    # --- shared, once per core ------------------------------------------
    ident = sh.tile([P, P], BF16)
    make_identity(nc, ident[:])

    qdt32 = sh.tile([P, NT, M], F32)   # Qd^T  [k, m] as [p, t, m]
    nc.sync.dma_start(qdt32[:], io["QdT32"].rearrange("(t p) m -> p t m", p=P))
    qdt16 = sh.tile([P, NT, M], BF16)
    nc.sync.dma_start(qdt16[:], io["QdT16"].rearrange("(t p) m -> p t m", p=P))
    qdn32 = sh.tile([P, MT, N], F32)   # Qd    [m, q] as [p, c, q]
    nc.sync.dma_start(qdn32[:], io["QdN32"].rearrange("(c p) q -> p c q", p=P))

    for h in range(HPC):
        # --- per-head input prep ---------------------------------------
        mk = msk_pool.tile([P, NT], F32)
        nc.sync.dma_start(mk[:], io["maskT"][h])

        qt = qt_pool.tile([64, N], BF16)           # Q^T [d, q]
        nc.sync.dma_start(qt[:], io["QT"][h])

        vm = v_pool.tile([P, NT, D], F32)          # V natural [k, d]
        nc.sync.dma_start(vm[:], io["V"][h].rearrange("(t p) d -> p t d", p=P))
        for t in range(NT):
            nc.vector.tensor_scalar_mul(vm[:, t, :], vm[:, t, :], mk[:, t : t + 1])

        kts = kt_pool.tile([64, N], BF16)          # (K*mask)^T [d, k]
        k_r = io["K"][h].rearrange("(t p) d -> t p d", p=P)
        for t in range(NT):
            kl = kld_pool.tile([P, D], BF16, tag="kld")
            nc.sync.dma_start(kl[:], k_r[t])
            nc.vector.tensor_scalar_mul(kl[:], kl[:], mk[:, t : t + 1])
            pt = ps_m.tile([D, P], BF16, tag="misc")
            nc.tensor.transpose(pt[:], kl[:], ident[:])
            nc.vector.tensor_copy(kts[:, t * P : (t + 1) * P], pt[:])

        # --- Vd = Qd @ (V*m)  -> [M, D] --------------------------------
        vd = vd_pool.tile([P, MT, D], F32)
        for mh in range(MT):
            vps = ps_m.tile([P, D], F32, tag="misc")
            for t in range(NT):
                nc.tensor.matmul(
                    vps[:],
                    lhsT=qdt32[:, t, mh * P : (mh + 1) * P],
                    rhs=vm[:, t, :],
                    start=(t == 0),
                    stop=(t == NT - 1),
                )
            nc.vector.tensor_copy(vd[:, mh, :], vps[:])

        # --- phase A: scores -> exp (unnormalized) + CqT ----------------
        exps = []
        cq = cq_pool.tile([P, NT, M], BF16)
        for q in range(NT):
            ex = exp_pool.tile([P, N], BF16, tag="exp")
            sums = st_pool.tile([P, KC], F32, tag="sums")
            for c in range(KC):
                sps = ps_s.tile([P, 512], F32, tag="s")
                nc.tensor.matmul(
                    sps[:],
                    lhsT=qt[:, q * P : (q + 1) * P],
                    rhs=kts[:, c * 512 : (c + 1) * 512],
                    start=True,
                    stop=True,
                )
                nc.scalar.activation(
                    ex[:, c * 512 : (c + 1) * 512],
                    sps[:],
                    AF.Exp,
                    scale=0.125,
                    accum_out=sums[:, c : c + 1],
                )
            den = st_pool.tile([P, 1], F32, tag="den")
            nc.vector.tensor_reduce(den[:], sums[:], axis=AX.X, op=ALU.add)
            rec = st_pool.tile([P, 1], F32, tag="rec")
            nc.vector.reciprocal(rec[:], den[:])
            nc.vector.tensor_scalar_mul(cq[:, q, :], qdt32[:, q, :], rec[:])
            exps.append(ex)

        # --- phase B: A1^T[k,m] = sum_q exp[q,k] * CqT[q,m] -------------
        a1 = a1_pool.tile([P, NT, M], BF16)
        for kc in range(NT):
            aps_ = ps_a1.tile([P, M], F32, tag="a1")
            for q in range(NT):
                nc.tensor.matmul(
                    aps_[:],
                    lhsT=exps[q][:, kc * P : (kc + 1) * P],
                    rhs=cq[:, q, :],
                    start=(q == 0),
                    stop=(q == NT - 1),
                )
            nc.vector.tensor_copy(a1[:, kc, :], aps_[:])

        # --- G^T[n,m] = sum_k QdT[k,n] * A1T[k,m] -----------------------
        gt = gt_pool.tile([P, MT, M], F32)
        for nh in range(MT):
            gps = ps_gt.tile([P, M], F32, tag="g")
            for kc in range(NT):
                nc.tensor.matmul(
                    gps[:],
                    lhsT=qdt16[:, kc, nh * P : (nh + 1) * P],
                    rhs=a1[:, kc, :],
                    start=(kc == 0),
                    stop=(kc == NT - 1),
                )
            nc.vector.tensor_copy(gt[:, nh, :], gps[:])

        # --- y[m,d] = sum_n GT[n,m] * Vd[n,d] ---------------------------
        yt = y_pool.tile([P, MT, D], F32)
        for mh in range(MT):
            yps = ps_m.tile([P, D], F32, tag="misc")
            for nh in range(MT):
                nc.tensor.matmul(
                    yps[:],
                    lhsT=gt[:, nh, mh * P : (mh + 1) * P],
                    rhs=vd[:, nh, :],
                    start=(nh == 0),
                    stop=(nh == MT - 1),
                )
            nc.vector.tensor_copy(yt[:, mh, :], yps[:])

        # --- out[q,d] = sum_m Qd[m,q] * y[m,d] --------------------------
        o_r = io["out"][h].rearrange("(t p) d -> t p d", p=P)
        for q in range(NT):
            ops_ = ps_m.tile([P, D], F32, tag="misc")
            for mh in range(MT):
                nc.tensor.matmul(
                    ops_[:],
                    lhsT=qdn32[:, mh, q * P : (q + 1) * P],
                    rhs=yt[:, mh, :],
                    start=(mh == 0),
                    stop=(mh == MT - 1),
                )
            ost = ost_pool.tile([P, D], F32, tag="ost")
            nc.vector.tensor_copy(ost[:], ops_[:])
            nc.sync.dma_start(o_r[q], ost[:])


def build_nc():
    from contextlib import ExitStack

    nc = bacc.Bacc("TRN2", target_bir_lowering=False, debug=False)
    io = {
        "QT": nc.dram_tensor("QT", [HPC, 64, N], BF16, kind="ExternalInput").ap(),
        "K": nc.dram_tensor("K", [HPC, N, D], BF16, kind="ExternalInput").ap(),
        "V": nc.dram_tensor("V", [HPC, N, D], F32, kind="ExternalInput").ap(),
        "maskT": nc.dram_tensor("maskT", [HPC, 128, NT], F32, kind="ExternalInput").ap(),
        "QdT32": nc.dram_tensor("QdT32", [N, M], F32, kind="ExternalInput").ap(),
        "QdT16": nc.dram_tensor("QdT16", [N, M], BF16, kind="ExternalInput").ap(),
        "QdN32": nc.dram_tensor("QdN32", [M, N], F32, kind="ExternalInput").ap(),
        "out": nc.dram_tensor("out", [HPC, N, D], F32, kind="ExternalOutput").ap(),
    }
    with tile.TileContext(nc) as tc:
        with ExitStack() as ctx:
            _emit(tc, ctx, io)
    nc.compile()
    return nc


_NC = None


def _get_nc():
    global _NC
    if _NC is None:
        _NC = build_nc()
    return _NC


def make_in_maps(Q, K, V, mask, Q_dct):
    Q = np.asarray(Q, dtype=np.float32).reshape(B * H, N, D)
    K = np.asarray(K, dtype=np.float32).reshape(B * H, N, D)
    V = np.asarray(V, dtype=np.float32).reshape(B * H, N, D)
    mask = np.asarray(mask, dtype=np.float32)
    Q_dct = np.asarray(Q_dct, dtype=np.float32)

    QT = np.ascontiguousarray(Q.transpose(0, 2, 1)).astype(NPBF16)
    K16 = K.astype(NPBF16)
    QdT32 = np.ascontiguousarray(Q_dct.T)
    QdT16 = QdT32.astype(NPBF16)
    QdN32 = np.ascontiguousarray(Q_dct)
    # maskT[h, p, t] = mask[b(h), t*128 + p]
    maskT = np.ascontiguousarray(
        mask.reshape(B, NT, 128).transpose(0, 2, 1)
    )  # [B, 128, NT]

    in_maps = []
    for c in range(NCORES):
        sl = slice(HPC * c, HPC * (c + 1))
        heads = range(HPC * c, HPC * (c + 1))
        in_maps.append(
            {
                "QT": np.ascontiguousarray(QT[sl]),
                "K": np.ascontiguousarray(K16[sl]),
                "V": np.ascontiguousarray(V[sl]),
                "maskT": np.ascontiguousarray(
                    np.stack([maskT[hp // H] for hp in heads])
                ),
                "QdT32": QdT32,
                "QdT16": QdT16,
                "QdN32": QdN32,
            }
        )
    return in_maps


def run_on_device(in_maps, **kwargs):
    nc = _get_nc()
    return bass_utils.run_bass_kernel_spmd(
        nc, in_maps, core_ids=list(range(NCORES)), **kwargs
    )


def kernel(Q, K, V, mask, Q_dct):
    in_maps = make_in_maps(Q, K, V, mask, Q_dct)
    res = run_on_device(in_maps)
    out = np.empty((B * H, N, D), dtype=np.float32)
    for c in range(NCORES):
        out[HPC * c : HPC * (c + 1)] = res.results[c]["out"]
    return out.reshape(B, H, N, D)
